# revision 10
# baseline (speedup 1.0000x reference)
"""Trainium2 Bass kernel for nn_AttentionHead (additive/Bahdanau attention).

reference:
    kt = einsum('bkh,oh->bko', x_key, w1)          # (B, NK, H)
    qt = einsum('bqh,oh->bqo', x_query, w2)        # (B, NQ, H)
    prod[b,q,k] = sum_h v[h] * tanh(kt[b,k,h] + qt[b,q,h])
    out = log_softmax(prod, axis=-1)               # (B, NQ, NK)

Shapes: B=4, NQ=256, NK=512, H=256.  8 NeuronCores, data-parallel over
(B x NQ/2): core c handles b = c//2 and a 128-row slice of NQ.

Per-core dataflow:
  - host marshals packed fp32 inputs: transposed xk, xq, w1, w2 plus the
    "ediag" stationaries (for each (h_tile, j in 0..31) a (128,32) matrix,
    zero except column j = v[h_tile*128 : +128]).
  - PE: ktT[o_t] (128, 512) = w1T.T @ xkT       (o on partitions, k free)
        qtT[o_t] (128, 128) = w2T.T @ xqT       (o on partitions, q free)
        ktT cast to bf16 on the PSUM->SBUF copy.
  - DVE: S[h_t][:, q*512:+512] = ktT[h_t] + qtT[h_t][:, q]  (bf16 in/out,
    fp32 per-partition scalar -> high DVE perf mode)
  - ACT: tanh in place on S in large (128, 8192) instructions (the kernel
    bottleneck: 134M tanh elements live on ScalarE only)
  - PE:  prod[q, :] += ediag[h_t, q%32].T @ S[h_t][:, q]  via col-tiled
    matmuls (tile_position=(0, 32j)) accumulating into one PSUM bank
    (q on partitions, k free)
  - log_softmax along free axis, regionized over 32-q row blocks so most
    of it overlaps the main loop: out = prod - ln(sum_k exp(prod));
    |prod| <= sum|v| ~ 8 so skipping max-subtraction is safe in fp32.

walrus only supports ONE sync wait per instruction: split_multi_waits()
post-processes the scheduled IR, moving extra waits onto same-engine
NoOps inserted immediately before the offending instruction.
"""

import sys

sys.path.insert(0, "/opt/trn_rl_repo")

import numpy as np

import concourse.bass as bass
import concourse.mybir as mybir
from concourse import tile
from concourse.bass_utils import run_bass_kernel_spmd

F32 = mybir.dt.float32
BF16 = mybir.dt.bfloat16
AF = mybir.ActivationFunctionType
ALU = mybir.AluOpType

B, NQ, NK, H = 4, 256, 512, 256
NCORES = 8
QPC = (B * NQ) // NCORES  # 128 q rows per core
GROUP = 16                # q's per pipeline group
NGROUPS = QPC // GROUP    # 8

OFF_XKT = 0               # 2 x (128, 512)
OFF_XQT = 1024            # 2 x (128, 128)
OFF_W1T = 1280            # 2 x (128, 256)
OFF_W2T = 1792            # 2 x (128, 256)
PACKED_F = 2304
ED_F = 2 * 32 * 32        # separate (128, 2048) fp32 input


def build_program(split=True):
    nc = bass.Bass()

    pk_d = nc.dram_tensor("packed", (128, PACKED_F), F32, kind="ExternalInput")
    ed_d = nc.dram_tensor("ediag", (128, ED_F), F32, kind="ExternalInput")
    out_d = nc.dram_tensor("out", (QPC, NK), F32, kind="ExternalOutput")

    with tile.TileContext(nc) as tc:
        with (
            tc.tile_pool(name="const", bufs=1) as cpool,
            tc.tile_pool(name="sadd", bufs=2) as spool,
            tc.tile_pool(name="ppre", bufs=2, space="PSUM") as ppool,
            tc.tile_pool(name="prod", bufs=1, space="PSUM") as prodpool,
        ):
            packed = cpool.tile([128, PACKED_F], F32, tag="packed")
            nc.sync.dma_start(packed[:], pk_d[:])
            ed_f32 = cpool.tile([128, ED_F], F32, tag="ed_f32")
            nc.sync.dma_start(ed_f32[:], ed_d[:])

            def xkT(i):
                return packed[:, OFF_XKT + i * NK:OFF_XKT + (i + 1) * NK]

            def xqT(i):
                return packed[:, OFF_XQT + i * QPC:OFF_XQT + (i + 1) * QPC]

            def w1T(i, o):
                return packed[:, OFF_W1T + i * 256 + o * 128:OFF_W1T + i * 256 + (o + 1) * 128]

            def w2T(i, o):
                return packed[:, OFF_W2T + i * 256 + o * 128:OFF_W2T + i * 256 + (o + 1) * 128]

            # v-diag stationaries -> bf16 (ACT cast keeps main-loop matmul
            # deps on the single ACT semaphore)
            ed_bf = cpool.tile([128, ED_F], BF16, tag="ed_bf")
            nc.scalar.copy(ed_bf[:], ed_f32[:])

            # ---- ktT / qtT ---------------------------------------------------
            ktT_sb = [cpool.tile([128, NK], BF16, tag=f"ktT{o}", name=f"ktT{o}")
                      for o in range(2)]
            qtT_sb = [cpool.tile([128, QPC], F32, tag=f"qtT{o}", name=f"qtT{o}")
                      for o in range(2)]
            for o_t in range(2):
                pk = ppool.tile([128, NK], F32, tag="pk", name="pk")
                for h_t in range(2):
                    nc.tensor.matmul(
                        pk[:], w1T(h_t, o_t), xkT(h_t),
                        start=(h_t == 0), stop=(h_t == 1),
                    )
                nc.vector.tensor_copy(ktT_sb[o_t][:], pk[:])
            for o_t in range(2):
                pq = ppool.tile([128, QPC], F32, tag="pq", name="pq")
                for h_t in range(2):
                    nc.tensor.matmul(
                        pq[:], w2T(h_t, o_t), xqT(h_t),
                        start=(h_t == 0), stop=(h_t == 1),
                    )
                nc.vector.tensor_copy(qtT_sb[o_t][:], pq[:])

            # ---- main loop ---------------------------------------------------
            # prod accumulates in one PSUM bank; 32-q row regions complete at
            # even group boundaries, so their softmax overlaps the main loop.
            prod = prodpool.tile([128, NK], F32)
            sumexp = cpool.tile([128, 1], F32, tag="sumexp")
            lse = cpool.tile([128, 1], F32, tag="lse")
            neg_lse = cpool.tile([128, 1], F32, tag="neg_lse")
            expt = cpool.tile([128, NK], F32, tag="expt")
            out_sb = cpool.tile([128, NK], F32, tag="out_sb")

            for g in range(NGROUPS):
                S = [spool.tile([128, GROUP * NK], BF16, tag=f"S{i}", name=f"S{i}")
                     for i in range(2)]
                for ql in range(GROUP):
                    q = g * GROUP + ql
                    for h_t in range(2):
                        nc.vector.tensor_scalar(
                            S[h_t][:, ql * NK:(ql + 1) * NK],
                            ktT_sb[h_t][:],
                            qtT_sb[h_t][:, q:q + 1],
                            None,
                            op0=ALU.add,
                        )
                for h_t in range(2):
                    nc.scalar.activation(S[h_t][:], S[h_t][:], AF.Tanh)
                for h_t in range(2):
                    for ql in range(GROUP):
                        q = g * GROUP + ql
                        j = (q // 32) * 32
                        jj = q % 32
                        nc.tensor.matmul(
                            prod[j:j + 32, :],
                            ed_bf[:, h_t * 1024 + jj * 32: h_t * 1024 + jj * 32 + 32],
                            S[h_t][:, ql * NK:(ql + 1) * NK],
                            start=(jj == 0 and h_t == 0),
                            stop=(jj == 31 and h_t == 1),
                            tile_position=(0, j),
                        )
                if g % 2 == 1:
                    # region r = 32-q row block finished by this group
                    r = g // 2
                    rows = slice(r * 32, (r + 1) * 32)
                    nc.scalar.activation(
                        expt[rows, :], prod[rows, :], AF.Exp,
                        accum_out=sumexp[rows, :],
                    )
                    nc.scalar.activation(lse[rows, :], sumexp[rows, :], AF.Ln)
                    nc.vector.tensor_scalar_mul(neg_lse[rows, :], lse[rows, :], -1.0)
                    nc.scalar.activation(
                        out_sb[rows, :], prod[rows, :], AF.Identity,
                        bias=neg_lse[rows, 0:1],
                    )
                    nc.sync.dma_start(out_d[rows, :], out_sb[rows, :])

    if split:
        split_multi_waits(nc)
    return nc


def split_multi_waits(nc):
    """walrus codegen accepts at most one sync wait per instruction; move
    extra waits onto same-engine NoOps inserted immediately before."""
    n = 0
    for fn in nc.m.functions:
        for blk in fn.blocks:
            new_insts = []
            for inst in blk.instructions:
                si = inst.sync_info
                if si is not None and len(si.on_wait) > 1:
                    waits = list(si.on_wait)
                    for w in waits[:-1]:
                        nop = mybir.InstNoOp(name=f"WSPLIT-{n}", ins=[], outs=[])
                        n += 1
                        nop.engine = inst.engine
                        nop.sync_info = mybir.SyncInfo(on_wait=[w], on_update=[])
                        new_insts.append(nop)
                    inst.sync_info = mybir.SyncInfo(
                        on_wait=[waits[-1]], on_update=list(si.on_update)
                    )
                new_insts.append(inst)
            if n:
                blk.instructions = new_insts
    return n


def audit_waits(nc, max_waits=1):
    bad = []
    for fn in nc.m.functions:
        for blk in fn.blocks:
            for inst in blk.instructions:
                si = inst.sync_info
                if si is not None and len(si.on_wait) > max_waits:
                    bad.append((inst.name, type(inst).__name__,
                                [w.ant_name for w in si.on_wait]))
    return bad


def make_in_maps(x_query, x_key, w1, w2, v):
    x_query = np.asarray(x_query, dtype=np.float32)
    x_key = np.asarray(x_key, dtype=np.float32)
    w1 = np.asarray(w1, dtype=np.float32)
    w2 = np.asarray(w2, dtype=np.float32)
    v = np.asarray(v, dtype=np.float32).reshape(H)

    w1T = np.ascontiguousarray(w1.T)  # (h_in, o)
    w2T = np.ascontiguousarray(w2.T)

    # ediag[p, h_t*1024 + j*32 + c] = v[h_t*128 + p] if c == j else 0
    ed = np.zeros((128, 2, 32, 32), dtype=np.float32)
    for h_t in range(2):
        for j in range(32):
            ed[:, h_t, j, j] = v[h_t * 128:(h_t + 1) * 128]
    ed = np.ascontiguousarray(ed.reshape(128, ED_F))

    in_maps = []
    for c in range(NCORES):
        b = c // 2
        q0 = (c % 2) * QPC
        xqT = np.ascontiguousarray(x_query[b, q0:q0 + QPC, :].T)  # (H, 128)
        xkT = np.ascontiguousarray(x_key[b].T)                    # (H, 512)
        packed = np.concatenate(
            [
                xkT[:128], xkT[128:],
                xqT[:128], xqT[128:],
                w1T[:128], w1T[128:],
                w2T[:128], w2T[128:],
            ],
            axis=1,
        )
        assert packed.shape == (128, PACKED_F)
        in_maps.append({
            "packed": np.ascontiguousarray(packed),
            "ediag": ed,
        })
    return in_maps


_prog_cache = {}


def kernel(x_query, x_key, w1, w2, v):
    if "nc" not in _prog_cache:
        _prog_cache["nc"] = build_program()
    nc = _prog_cache["nc"]
    in_maps = make_in_maps(x_query, x_key, w1, w2, v)
    res = run_bass_kernel_spmd(nc, in_maps, list(range(NCORES)))
    out = np.empty((B, NQ, NK), dtype=np.float32)
    for c in range(NCORES):
        b = c // 2
        q0 = (c % 2) * QPC
        out[b, q0:q0 + QPC, :] = res.results[c]["out"]
    return out


if __name__ == "__main__":
    nc = build_program()
    bad = audit_waits(nc)
    if bad:
        print(f"{len(bad)} instructions exceed the 1-wait budget:")
        for name, ty, waits in bad[:20]:
            print(" ", name, ty, waits)
    else:
        print("wait audit OK: all instructions <= 1 sync wait")


# revision 11
# speedup vs baseline: 1.3685x; 1.3685x over previous
"""Trainium2 Bass kernel for nn_AttentionHead (additive/Bahdanau attention).

reference:
    kt = einsum('bkh,oh->bko', x_key, w1)          # (B, NK, H)
    qt = einsum('bqh,oh->bqo', x_query, w2)        # (B, NQ, H)
    prod[b,q,k] = sum_h v[h] * tanh(kt[b,k,h] + qt[b,q,h])
    out = log_softmax(prod, axis=-1)               # (B, NQ, NK)

Shapes: B=4, NQ=256, NK=512, H=256.  8 NeuronCores, data-parallel over
(B x NQ/2): core c handles b = c//2 and a 128-row slice of NQ.

Per-core dataflow:
  - host marshals packed fp32 inputs: transposed xk, xq, w1, w2 plus the
    "ediag" stationaries (for each (h_tile, j in 0..31) a (128,32) matrix,
    zero except column j = v[h_tile*128 : +128]).
  - PE: ktT[o_t] (128, 512) = w1T.T @ xkT       (o on partitions, k free)
        qtT[o_t] (128, 128) = w2T.T @ xqT       (o on partitions, q free)
        ktT cast to bf16 on the PSUM->SBUF copy.
  - DVE: S[h_t][:, q*512:+512] = ktT[h_t] + qtT[h_t][:, q]  (bf16 in/out,
    fp32 per-partition scalar -> high DVE perf mode)
  - ACT: tanh in place on S in large (128, 8192) instructions (the kernel
    bottleneck: 134M tanh elements live on ScalarE only)
  - PE:  prod[q, :] += ediag[h_t, q%32].T @ S[h_t][:, q]  via col-tiled
    matmuls (tile_position=(0, 32j)) accumulating into one PSUM bank
    (q on partitions, k free)
  - log_softmax along free axis, regionized over 32-q row blocks so most
    of it overlaps the main loop: out = prod - ln(sum_k exp(prod));
    |prod| <= sum|v| ~ 8 so skipping max-subtraction is safe in fp32.

walrus only supports ONE sync wait per instruction: split_multi_waits()
post-processes the scheduled IR, moving extra waits onto same-engine
NoOps inserted immediately before the offending instruction.
"""

import sys

sys.path.insert(0, "/opt/trn_rl_repo")

import numpy as np
import ml_dtypes

import concourse.bass as bass
import concourse.mybir as mybir
from concourse import tile
from concourse.bass_utils import run_bass_kernel_spmd

F32 = mybir.dt.float32
BF16 = mybir.dt.bfloat16
AF = mybir.ActivationFunctionType
ALU = mybir.AluOpType

B, NQ, NK, H = 4, 256, 512, 256
NCORES = 8
QPC = (B * NQ) // NCORES  # 128 q rows per core
GROUP = 16                # q's per pipeline group
NGROUPS = QPC // GROUP    # 8

OFF_XKT = 0               # 2 x (128, 512)
OFF_XQT = 1024            # 2 x (128, 128)
OFF_W1T = 1280            # 2 x (128, 256)
OFF_W2T = 1792            # 2 x (128, 256)
PACKED_F = 2304
ED_F = 2 * 32 * 32        # separate (128, 2048) fp32 input


def build_program(split=True):
    nc = bass.Bass()

    pk_d = nc.dram_tensor("packed", (128, PACKED_F), BF16, kind="ExternalInput")
    ed_d = nc.dram_tensor("ediag", (128, ED_F), BF16, kind="ExternalInput")
    out_d = nc.dram_tensor("out", (QPC, NK), F32, kind="ExternalOutput")

    with tile.TileContext(nc) as tc:
        with (
            tc.tile_pool(name="const", bufs=1) as cpool,
            tc.tile_pool(name="sadd", bufs=2) as spool,
            tc.tile_pool(name="ppre", bufs=2, space="PSUM") as ppool,
            tc.tile_pool(name="prod", bufs=1, space="PSUM") as prodpool,
        ):
            packed = cpool.tile([128, PACKED_F], BF16, tag="packed")
            nc.sync.dma_start(packed[:], pk_d[:])
            ed_bf = cpool.tile([128, ED_F], BF16, tag="ed_bf")
            nc.sync.dma_start(ed_bf[:], ed_d[:])

            def xkT(i):
                return packed[:, OFF_XKT + i * NK:OFF_XKT + (i + 1) * NK]

            def xqT(i):
                return packed[:, OFF_XQT + i * QPC:OFF_XQT + (i + 1) * QPC]

            def w1T(i, o):
                return packed[:, OFF_W1T + i * 256 + o * 128:OFF_W1T + i * 256 + (o + 1) * 128]

            def w2T(i, o):
                return packed[:, OFF_W2T + i * 256 + o * 128:OFF_W2T + i * 256 + (o + 1) * 128]

            # ---- ktT / qtT ---------------------------------------------------
            ktT_sb = [cpool.tile([128, NK], BF16, tag=f"ktT{o}", name=f"ktT{o}")
                      for o in range(2)]
            qtT_sb = [cpool.tile([128, QPC], F32, tag=f"qtT{o}", name=f"qtT{o}")
                      for o in range(2)]
            for o_t in range(2):
                pk = ppool.tile([128, NK], F32, tag="pk", name="pk")
                for h_t in range(2):
                    nc.tensor.matmul(
                        pk[:], w1T(h_t, o_t), xkT(h_t),
                        start=(h_t == 0), stop=(h_t == 1),
                    )
                nc.vector.tensor_copy(ktT_sb[o_t][:], pk[:])
            for o_t in range(2):
                pq = ppool.tile([128, QPC], F32, tag="pq", name="pq")
                for h_t in range(2):
                    nc.tensor.matmul(
                        pq[:], w2T(h_t, o_t), xqT(h_t),
                        start=(h_t == 0), stop=(h_t == 1),
                    )
                nc.vector.tensor_copy(qtT_sb[o_t][:], pq[:])

            # ---- main loop ---------------------------------------------------
            # prod accumulates in one PSUM bank; 32-q row regions complete at
            # even group boundaries, so their softmax overlaps the main loop.
            prod = prodpool.tile([128, NK], F32)
            sumexp = cpool.tile([128, 1], F32, tag="sumexp")
            lse = cpool.tile([128, 1], F32, tag="lse")
            neg_lse = cpool.tile([128, 1], F32, tag="neg_lse")
            expt = cpool.tile([128, NK], F32, tag="expt")
            out_sb = cpool.tile([128, NK], F32, tag="out_sb")

            for g in range(NGROUPS):
                S = [spool.tile([128, GROUP * NK], BF16, tag=f"S{i}", name=f"S{i}")
                     for i in range(2)]
                for h_t in range(2):
                    for ql in range(GROUP):
                        q = g * GROUP + ql
                        nc.vector.tensor_scalar(
                            S[h_t][:, ql * NK:(ql + 1) * NK],
                            ktT_sb[h_t][:],
                            qtT_sb[h_t][:, q:q + 1],
                            None,
                            op0=ALU.add,
                        )
                for h_t in range(2):
                    nc.scalar.activation(S[h_t][:], S[h_t][:], AF.Tanh)
                for h_t in range(2):
                    for ql in range(GROUP):
                        q = g * GROUP + ql
                        j = (q // 32) * 32
                        jj = q % 32
                        nc.tensor.matmul(
                            prod[j:j + 32, :],
                            ed_bf[:, h_t * 1024 + jj * 32: h_t * 1024 + jj * 32 + 32],
                            S[h_t][:, ql * NK:(ql + 1) * NK],
                            start=(jj == 0 and h_t == 0),
                            stop=(jj == 31 and h_t == 1),
                            tile_position=(0, j),
                        )

            # ---- log_softmax over k (free axis) ------------------------------
            nc.scalar.activation(expt[:], prod[:], AF.Exp, accum_out=sumexp[:])
            nc.scalar.activation(lse[:], sumexp[:], AF.Ln)
            nc.vector.tensor_scalar_mul(neg_lse[:], lse[:], -1.0)
            nc.scalar.activation(
                out_sb[:], prod[:], AF.Identity, bias=neg_lse[:, 0:1]
            )
            nc.sync.dma_start(out_d[:], out_sb[:])

    if split:
        split_multi_waits(nc)
    return nc


def split_multi_waits(nc):
    """walrus codegen accepts at most one sync wait per instruction; move
    extra waits onto same-engine NoOps inserted immediately before."""
    n = 0
    for fn in nc.m.functions:
        for blk in fn.blocks:
            new_insts = []
            for inst in blk.instructions:
                si = inst.sync_info
                if si is not None and len(si.on_wait) > 1:
                    waits = list(si.on_wait)
                    for w in waits[:-1]:
                        nop = mybir.InstNoOp(name=f"WSPLIT-{n}", ins=[], outs=[])
                        n += 1
                        nop.engine = inst.engine
                        nop.sync_info = mybir.SyncInfo(on_wait=[w], on_update=[])
                        new_insts.append(nop)
                    inst.sync_info = mybir.SyncInfo(
                        on_wait=[waits[-1]], on_update=list(si.on_update)
                    )
                new_insts.append(inst)
            if n:
                blk.instructions = new_insts
    return n


def audit_waits(nc, max_waits=1):
    bad = []
    for fn in nc.m.functions:
        for blk in fn.blocks:
            for inst in blk.instructions:
                si = inst.sync_info
                if si is not None and len(si.on_wait) > max_waits:
                    bad.append((inst.name, type(inst).__name__,
                                [w.ant_name for w in si.on_wait]))
    return bad


def make_in_maps(x_query, x_key, w1, w2, v):
    x_query = np.asarray(x_query, dtype=np.float32)
    x_key = np.asarray(x_key, dtype=np.float32)
    w1 = np.asarray(w1, dtype=np.float32)
    w2 = np.asarray(w2, dtype=np.float32)
    v = np.asarray(v, dtype=np.float32).reshape(H)

    w1T = np.ascontiguousarray(w1.T)  # (h_in, o)
    w2T = np.ascontiguousarray(w2.T)

    # ediag[p, h_t*1024 + j*32 + c] = v[h_t*128 + p] if c == j else 0
    ed = np.zeros((128, 2, 32, 32), dtype=np.float32)
    for h_t in range(2):
        for j in range(32):
            ed[:, h_t, j, j] = v[h_t * 128:(h_t + 1) * 128]
    ed = np.ascontiguousarray(ed.reshape(128, ED_F).astype(ml_dtypes.bfloat16))

    in_maps = []
    for c in range(NCORES):
        b = c // 2
        q0 = (c % 2) * QPC
        xqT = np.ascontiguousarray(x_query[b, q0:q0 + QPC, :].T)  # (H, 128)
        xkT = np.ascontiguousarray(x_key[b].T)                    # (H, 512)
        packed = np.concatenate(
            [
                xkT[:128], xkT[128:],
                xqT[:128], xqT[128:],
                w1T[:128], w1T[128:],
                w2T[:128], w2T[128:],
            ],
            axis=1,
        )
        assert packed.shape == (128, PACKED_F)
        in_maps.append({
            "packed": np.ascontiguousarray(packed.astype(ml_dtypes.bfloat16)),
            "ediag": ed,
        })
    return in_maps


_prog_cache = {}


def kernel(x_query, x_key, w1, w2, v):
    if "nc" not in _prog_cache:
        _prog_cache["nc"] = build_program()
    nc = _prog_cache["nc"]
    in_maps = make_in_maps(x_query, x_key, w1, w2, v)
    res = run_bass_kernel_spmd(nc, in_maps, list(range(NCORES)))
    out = np.empty((B, NQ, NK), dtype=np.float32)
    for c in range(NCORES):
        b = c // 2
        q0 = (c % 2) * QPC
        out[b, q0:q0 + QPC, :] = res.results[c]["out"]
    return out


if __name__ == "__main__":
    nc = build_program()
    bad = audit_waits(nc)
    if bad:
        print(f"{len(bad)} instructions exceed the 1-wait budget:")
        for name, ty, waits in bad[:20]:
            print(" ", name, ty, waits)
    else:
        print("wait audit OK: all instructions <= 1 sync wait")
